# revision 7
# baseline (speedup 1.0000x reference)
"""Multi-head causal attention (B=8, S=1024, E=512, H=8, Dk=Dv=64) on 8 NeuronCores.

Sharding: data-parallel over batch. Core b computes the full attention block
for X[b]; no collectives. Host pre-transposes X[b] -> [E, S], converts matmul
operands to bf16, and pre-arranges weights so the device kernel is pure
matmul + softmax.

v2 structure (vs v1):
  - PE warmup matmuls at t=0 so the HAM clock-gate goes 8/8 during input DMA.
  - Input DMAs reordered by first use and spread across 4 engine queues.
  - Every K=128 contraction is split into two concurrent K=64 row-tiled
    matmuls (tile_position (0,0)/(64,0)) so LDWEIGHTS overlaps the other
    half's in-flight matmul.
  - Causal diagonal: no PE mask matmuls; the exp'd triangle is zeroed by one
    DVE multiply with a 0/1 slab per diagonal block.
  - PSUM: tag "st" 2x[128,1024] double-buffered (projections, scores, and
    output-projection pairs all share it), tag "ot" 3x[65,512] so successive
    head-pairs' AV accumulations overlap the normalize chain.
  - qc-outer loop: output projection for the first 512 queries is emitted
    right after the qc=0 pairs and overlaps the qc=1 attention (ACT-bound).

Per-core dataflow (bf16 matmuls, fp32 PSUM accumulate / softmax math):
  V  = (X @ Wv + bv) with a ones column per head -> 4 tiles [128 s2, 2x8x65]
  QT/KT = (W^T X)^T per head-pair -> [128 dd, 1024 q]
  per head-pair, q-chunk (512 cols), k-block (128 rows):
    scores^T [128 k, 2x(512-off)] via two row-tiled K=64 matmuls,
    exp on ScalarE (scale=1/8), DVE triangle zeroing on diagonal blocks,
    O^T accum = V-slice^T @ exp (65th row = softmax denominator),
    O^T *= 1/denom via fast-NR reciprocal + gpsimd partition broadcast.
  Y[2 s-chunks] = sum_p O^T-block^T @ Wo_p + bo, one DMA per 256 rows.
"""

import numpy as np
import ml_dtypes

import concourse.bass as bass
import concourse.tile as tile
import concourse.mybir as mybir
from concourse import bacc
from concourse import bass_utils

B, S, E = 8, 1024, 512
H, DK, DV = 8, 64, 64
HD = H * DK  # 512
P = 128
EC = E // P  # 4 contraction chunks over E
NPAIR = H // 2
NCORES = 8
RSPLIT = False
WARMUP = True
YDMA3 = True
F32 = mybir.dt.float32
BF16 = mybir.dt.bfloat16

_COMPILED = None


def _mm_rsplit(nc, out, lhsT, rhs, start, stop):
    if RSPLIT:
        for rh in range(2):
            nc.tensor.matmul(out, lhsT[rh * 64:(rh + 1) * 64], rhs[rh * 64:(rh + 1) * 64],
                             start=(start and rh == 0), stop=(stop and rh == 1),
                             tile_position=(rh * 64, 0), skip_group_check=True)
    else:
        nc.tensor.matmul(out, lhsT, rhs, start=start, stop=stop, skip_group_check=True)


def _body(nc, tc, const, work, ps, d):
    # ---- PE warmup: keep TensorE busy from t=0 so the HAM clock-gate
    # flips to 8/8 while inputs stream in ----
    if WARMUP:
        wu_src = const.tile([P, 512], BF16, tag="wusrc", name="wu_src")
        nc.vector.memset(wu_src[:], 0.125)
        for i in range(8):
            wu = ps.tile([P, 512], F32, tag="wu", bufs=1, name=f"wu{i}")
            nc.tensor.matmul(wu[:, 0:P], wu_src[:, 0:P], wu_src[:, 0:P],
                             start=True, stop=True, skip_group_check=True)

    # ---- SBUF tiles for inputs ----
    xt = [const.tile([P, S], BF16, tag=f"xt{c}", name=f"xt{c}") for c in range(EC)]
    wv_sb = [const.tile([P, HD], BF16, tag=f"wv{c}", name=f"wv{c}") for c in range(EC)]
    wq_sb = [const.tile([P, HD], BF16, tag=f"wq{c}", name=f"wq{c}") for c in range(EC)]
    wk_sb = [const.tile([P, HD], BF16, tag=f"wk{c}", name=f"wk{c}") for c in range(EC)]
    wo_sb = [const.tile([P, E], BF16, tag=f"wo{c}", name=f"wo{c}") for c in range(EC)]
    bq_t = const.tile([P, NPAIR], F32, tag="bq", name="bq_t")
    bk_t = const.tile([P, NPAIR], F32, tag="bk", name="bk_t")
    bvb2_t = const.tile([P, 2 * HD], BF16, tag="bvb2", name="bvb2_t")
    bob2_t = const.tile([P, 2 * E], F32, tag="bob2", name="bob2_t")
    tri2_t = const.tile([P, 2 * P], BF16, tag="tri2", name="tri2_t")

    # ---- input DMAs: ordered by first use, spread over 4 queues ----
    for c in range(EC):
        nc.sync.dma_start(xt[c][:], d["xt"][c * P:(c + 1) * P, :])
        nc.sync.dma_start(wv_sb[c][:], d["wv"][c * P:(c + 1) * P, :])
    nc.sync.dma_start(bq_t[:], d["bq"][:])
    nc.sync.dma_start(bk_t[:], d["bk"][:])
    for c in range(EC):
        nc.sync.dma_start(wq_sb[c][:], d["wq"][c * P:(c + 1) * P, :])
    nc.sync.dma_start(bvb2_t[:], d["bvb2"][:])
    for c in range(EC):
        nc.sync.dma_start(wk_sb[c][:], d["wk"][c * P:(c + 1) * P, :])
    nc.sync.dma_start(tri2_t[:], d["tri2"][:])
    for c in range(EC):
        nc.sync.dma_start(wo_sb[c][:], d["wo"][c * P:(c + 1) * P, :])
    nc.sync.dma_start(bob2_t[:], d["bob2"][:])

    # ---- V = X @ Wv + bv, two s-chunks per PSUM pair-tile, augmented with a
    # ones column per head so the AV matmul also emits softmax denominators ----
    vd = []
    for j in range(4):
        vp = ps.tile([P, 1024], F32, tag="st", name=f"vp{j}")
        for sh in range(2):
            si = 2 * j + sh
            for c in range(EC):
                _mm_rsplit(nc, vp[:, sh * 512:(sh + 1) * 512],
                           xt[c][:, si * P:(si + 1) * P], wv_sb[c][:],
                           c == 0, c == EC - 1)
        t = const.tile([P, 2 * 520], BF16, tag=f"vd{j}", name=f"vd{j}")
        nc.vector.memset(t[:], 1.0)  # contiguous; leaves the per-head ones columns
        for sh in range(2):
            t3 = t[:, sh * 520:(sh + 1) * 520].rearrange("p (h c) -> p h c", c=65)
            nc.vector.tensor_add(
                t3[:, :, 0:DV],
                vp[:, sh * 512:(sh + 1) * 512].rearrange("p (h c) -> p h c", c=DV),
                bvb2_t[:, sh * 512:(sh + 1) * 512].rearrange("p (h c) -> p h c", c=DV))
        vd.append(t)

    # ---- QT / KT per head-pair: [128 dd, 1024 q] ----
    qt = {}
    kt = {}
    for p in range(NPAIR):
        qp = ps.tile([P, 1024], F32, tag="st", name=f"qp{p}")
        kp = ps.tile([P, 1024], F32, tag="st", name=f"kp{p}")
        for qc in range(2):
            for c in range(EC):
                _mm_rsplit(nc, qp[:, qc * 512:(qc + 1) * 512],
                           wq_sb[c][:, p * P:(p + 1) * P],
                           xt[c][:, qc * 512:(qc + 1) * 512], c == 0, c == EC - 1)
            for c in range(EC):
                _mm_rsplit(nc, kp[:, qc * 512:(qc + 1) * 512],
                           wk_sb[c][:, p * P:(p + 1) * P],
                           xt[c][:, qc * 512:(qc + 1) * 512], c == 0, c == EC - 1)
        qtt = const.tile([P, 1024], BF16, tag=f"qt{p}", name=f"qt{p}")
        nc.scalar.add(qtt[:], qp[:], bq_t[:, p:p + 1])
        ktt = const.tile([P, 1024], BF16, tag=f"kt{p}", name=f"kt{p}")
        nc.vector.tensor_scalar_add(ktt[:], kp[:], bk_t[:, p:p + 1])
        qt[p] = qtt
        kt[p] = ktt

    # ---- attention per q-chunk (outer) and head-pair (inner), then the
    # output projection for that q-chunk so it overlaps the next chunk ----
    ot_sb = {}
    for qc in range(2):
        for p in range(NPAIR):
            n_ki = 4 * (qc + 1)  # causal: skip k-blocks above the diagonal
            otp = {}
            for hb in range(2):
                otp[hb] = ps.tile([DV + 1, 512], F32, tag="ot", bufs=3,
                                  name=f"otp{p}_{qc}_{hb}")
            for ki in range(n_ki):
                diag = (ki * P - qc * 512) >= 0
                off = max(ki * P - qc * 512, 0)
                stp = ps.tile([P, 1024], F32, tag="st", name=f"st{p}_{qc}_{ki}")
                for hb in range(2):
                    hp = slice(hb * DK, (hb + 1) * DK)
                    nc.tensor.matmul(
                        stp[:, hb * 512 + off:(hb + 1) * 512],
                        kt[p][hp, ki * P:(ki + 1) * P],
                        qt[p][hp, qc * 512 + off:(qc + 1) * 512],
                        start=True, stop=True, tile_position=(hb * DK, 0),
                        skip_group_check=True)
                ste = work.tile([P, 1024], BF16, tag="ste", name=f"ste{p}_{qc}_{ki}")
                stp3 = stp.rearrange("p (h q) -> p h q", h=2)[:, :, off:]
                ste3 = ste.rearrange("p (h q) -> p h q", h=2)[:, :, off:]
                nc.scalar.activation(
                    ste3, stp3, mybir.ActivationFunctionType.Exp, scale=0.125)
                if diag:
                    # zero the exp'd causal triangle (cols off..off+128) for
                    # both heads in one DVE multiply
                    nc.vector.tensor_mul(
                        ste3[:, :, 0:P], ste3[:, :, 0:P],
                        tri2_t.rearrange("p (h q) -> p h q", h=2))
                st_f, sp_f = (ki == 0), (ki == n_ki - 1)
                for hb in range(2):
                    h = 2 * p + hb
                    vsl = (ki % 2) * 520 + h * 65
                    _mm_rsplit(nc, otp[hb][:, off:],
                               vd[ki // 2][:, vsl:vsl + 65],
                               ste[:, hb * 512 + off:(hb + 1) * 512], st_f, sp_f)
            ot = const.tile([P, 512], BF16, tag=f"ot{p}_{qc}", name=f"ot{p}_{qc}")
            for hb in range(2):
                h = 2 * p + hb
                rrow = work.tile([1, 512], F32, tag="rrow", name=f"rrow{h}_{qc}", bufs=2)
                nc.vector.tensor_copy(rrow[:], otp[hb][DV:DV + 1, :])
                rec = work.tile([1, 512], F32, tag="rec", name=f"rec{h}_{qc}", bufs=2)
                nc.vector.reciprocal_approx_fast(rec[:], rrow[:])
                rb = work.tile([DV, 512], F32, tag="rb", name=f"rb{h}_{qc}", bufs=2)
                nc.gpsimd.partition_broadcast(rb[:], rec[:])
                if hb == 0:
                    nc.vector.tensor_mul(ot[0:DV, :], otp[0][0:DV, :], rb[:])
                else:
                    # DVE cannot shift partitions: scale into a temp at base 0,
                    # then SBUF->SBUF DMA into partitions 64-127 of the pair tile
                    tmp = work.tile([DV, 512], BF16, tag="ottmp",
                                    name=f"ottmp{p}_{qc}", bufs=2)
                    nc.vector.tensor_mul(tmp[:], otp[1][0:DV, :], rb[:])
                    nc.sync.dma_start(ot[DV:P, :], tmp[:])
            ot_sb[p, qc] = ot

        # ---- output projection for this q-chunk: two s-chunks per PSUM tile ----
        for sj in range(2):
            yp = ps.tile([P, 1024], F32, tag="st", name=f"yp{qc}_{sj}")
            for sh in range(2):
                si = qc * 4 + sj * 2 + sh
                sl = si % 4
                for p in range(NPAIR):
                    _mm_rsplit(nc, yp[:, sh * 512:(sh + 1) * 512],
                               ot_sb[p, qc][:, sl * P:(sl + 1) * P],
                               wo_sb[p][:], p == 0, p == NPAIR - 1)
            yo = work.tile([P, 1024], F32, tag="yo", name=f"yo{qc}_{sj}", bufs=2)
            nc.vector.tensor_add(yo[:], yp[:], bob2_t[:])
            r0 = (qc * 4 + sj * 2) * P
            if YDMA3:
                yv = d["y"][r0:r0 + 2 * P, :].rearrange("(s p) e -> p s e", s=2)
                nc.sync.dma_start(yv, yo.rearrange("p (s e) -> p s e", s=2))
            else:
                nc.sync.dma_start(d["y"][r0:r0 + P, :], yo[:, 0:512])
                nc.sync.dma_start(d["y"][r0 + P:r0 + 2 * P, :], yo[:, 512:1024])


def _build():
    nc = bacc.Bacc("TRN2", target_bir_lowering=False, debug=False)
    d = {
        "xt": nc.dram_tensor("xt", [E, S], BF16, kind="ExternalInput").ap(),
        "wq": nc.dram_tensor("wq", [E, HD], BF16, kind="ExternalInput").ap(),
        "wk": nc.dram_tensor("wk", [E, HD], BF16, kind="ExternalInput").ap(),
        "wv": nc.dram_tensor("wv", [E, HD], BF16, kind="ExternalInput").ap(),
        "wo": nc.dram_tensor("wo", [HD, E], BF16, kind="ExternalInput").ap(),
        "tri2": nc.dram_tensor("tri2", [P, 2 * P], BF16, kind="ExternalInput").ap(),
        "bq": nc.dram_tensor("bq", [P, NPAIR], F32, kind="ExternalInput").ap(),
        "bk": nc.dram_tensor("bk", [P, NPAIR], F32, kind="ExternalInput").ap(),
        "bvb2": nc.dram_tensor("bvb2", [P, 2 * HD], BF16, kind="ExternalInput").ap(),
        "bob2": nc.dram_tensor("bob2", [P, 2 * E], F32, kind="ExternalInput").ap(),
        "y": nc.dram_tensor("y", [S, E], F32, kind="ExternalOutput").ap(),
    }
    with tile.TileContext(nc) as tc:
        with tc.tile_pool(name="const", bufs=1) as const, \
             tc.tile_pool(name="work", bufs=3) as work, \
             tc.tile_pool(name="ps", bufs=2, space="PSUM") as ps:
            _body(nc, tc, const, work, ps, d)
    nc.compile()
    return nc


def get_nc():
    global _COMPILED
    if _COMPILED is None:
        _COMPILED = _build()
    return _COMPILED


def _prep_in_maps(X, Wq, bq, Wk, bk, Wv, bv, Wo, bo):
    f = np.float32
    bf = ml_dtypes.bfloat16
    shared = {
        "wq": np.ascontiguousarray(
            np.transpose(np.asarray(Wq, f), (1, 0, 2)).reshape(E, HD).astype(bf)),
        "wk": np.ascontiguousarray(
            np.transpose(np.asarray(Wk, f), (1, 0, 2)).reshape(E, HD).astype(bf)),
        "wv": np.ascontiguousarray(
            np.transpose(np.asarray(Wv, f), (1, 0, 2)).reshape(E, HD).astype(bf)),
        "wo": np.ascontiguousarray(np.asarray(Wo, f).reshape(HD, E).astype(bf)),
        "bq": np.ascontiguousarray(np.asarray(bq, f).reshape(HD).reshape(NPAIR, P).T),
        "bk": np.ascontiguousarray(np.asarray(bk, f).reshape(HD).reshape(NPAIR, P).T),
        "bvb2": np.ascontiguousarray(np.tile(
            np.asarray(bv, f).reshape(1, HD), (P, 2)).astype(bf)),
        "bob2": np.ascontiguousarray(np.tile(np.asarray(bo, f).reshape(1, E), (P, 2))),
    }
    # 0/1 keep-mask for the diagonal 128x128 triangle (keep k <= q), twice
    # side by side so one DVE op covers both heads
    keep = np.triu(np.ones((P, P), dtype=f))
    shared["tri2"] = np.ascontiguousarray(np.tile(keep, (1, 2)).astype(bf))
    Xf = np.asarray(X, f)
    in_maps = []
    for b in range(B):
        m = dict(shared)
        m["xt"] = np.ascontiguousarray(Xf[b].T.astype(bf))
        in_maps.append(m)
    return in_maps


def kernel(X, Wq, bq, Wk, bk, Wv, bv, Wo, bo):
    nc = get_nc()
    in_maps = _prep_in_maps(X, Wq, bq, Wk, bk, Wv, bv, Wo, bo)
    res = bass_utils.run_bass_kernel_spmd(nc, in_maps, core_ids=list(range(NCORES)))
    return np.stack([res.results[b]["y"] for b in range(B)], axis=0).astype(np.float32)


def run_traced(X, Wq, bq, Wk, bk, Wv, bv, Wo, bo):
    """Like kernel() but with NTFF profiling; returns (out, exec_time_ns)."""
    nc = get_nc()
    in_maps = _prep_in_maps(X, Wq, bq, Wk, bk, Wv, bv, Wo, bo)
    res = bass_utils.run_bass_kernel_spmd(
        nc, in_maps, core_ids=list(range(NCORES)), trace=True)
    out = np.stack([res.results[b]["y"] for b in range(B)], axis=0).astype(np.float32)
    return out, res.exec_time_ns


# revision 9
# speedup vs baseline: 1.1379x; 1.1379x over previous
"""Multi-head causal attention (B=8, S=1024, E=512, H=8, Dk=Dv=64) on 8 NeuronCores.

Sharding: data-parallel over batch. Core b computes the full attention block
for X[b]; no collectives. Host pre-transposes X[b] -> [E, S], converts matmul
operands to bf16, and pre-arranges weights so the device kernel is pure
matmul + softmax.

v3 structure:
  - PE warmup matmuls at t=0 flip the HAM clock-gate to 8/8 during input DMA;
    input DMAs are ordered by first use and spread across sync/scalar/gpsimd
    descriptor queues.
  - The attention inner loop is ACT(exp)-bound, so projection work is
    interleaved INTO it as "filler units" (4 matmuls + one PSUM->SBUF convert
    each, all [128,512]) emitted between a k-block's scores and the previous
    block's AV matmuls.  This keeps TensorE dense (no HAM re-throttle) and
    software-pipelines the exp latency.
  - bv is folded into the output bias on the host (A@(V + 1 bv^T)/d = A@V/d +
    bv exactly, since the ones-column denominator divides out), so the V
    convert is a plain copy.
  - Causal diagonal: the exp'd triangle is zeroed by one DVE multiply with a
    0/1 slab per diagonal block (no PE mask matmuls).
  - PSUM: tag "st" 2x[128,1024] (scores double-buffer + upfront projection
    pairs), tag "ot" 3x[65,512] (AV accumulators overlap the normalize
    chain across head-pairs), tag "yw" 1x[128,512] (warmup + filler units).

Per-core dataflow (bf16 matmuls, fp32 PSUM accumulate / softmax math):
  V = X @ Wv with a ones column per head (AV emits softmax denominators)
  QT/KT = (W^T X)^T per head-pair -> [128 dd, 1024 q] (+bq/bk per partition)
  per head-pair, q-chunk, k-block: scores^T via two row-tiled K=64 matmuls,
    exp on ScalarE (scale=1/8), DVE triangle zero on diagonal blocks,
    O^T accum = V^T @ exp (65th row = denominator), O^T *= 1/denom via
    fast-NR reciprocal + gpsimd partition broadcast.
  Y[s-chunk] = sum_p O^T-chunk^T @ Wo_p + (bo + bv@Wo), one DMA per 128 rows.
"""

import numpy as np
import ml_dtypes

import concourse.bass as bass
import concourse.tile as tile
import concourse.mybir as mybir
from concourse import bacc
from concourse import bass_utils

B, S, E = 8, 1024, 512
H, DK, DV = 8, 64, 64
HD = H * DK  # 512
P = 128
EC = E // P  # 4 contraction chunks over E
NPAIR = H // 2
NCORES = 8
F32 = mybir.dt.float32
BF16 = mybir.dt.bfloat16

_COMPILED = None


def _body(nc, tc, const, work, ps, d):
    # ---- PE warmup: TensorE busy from t=0 so HAM goes 8/8 during input DMA ----
    wu_src = const.tile([P, 512], BF16, tag="wusrc", name="wu_src")
    nc.vector.memset(wu_src[:], 0.125)
    for i in range(7):
        wu = ps.tile([P, 512], F32, tag="yw", bufs=1, name=f"wu{i}")
        nc.tensor.matmul(wu[:, 0:P], wu_src[:, 0:P], wu_src[:, 0:P],
                         start=True, stop=True, skip_group_check=True)

    # ---- SBUF tiles for inputs ----
    xt = [const.tile([P, S], BF16, tag=f"xt{c}", name=f"xt{c}") for c in range(EC)]
    wv_sb = [const.tile([P, HD], BF16, tag=f"wv{c}", name=f"wv{c}") for c in range(EC)]
    wq_sb = [const.tile([P, HD], BF16, tag=f"wq{c}", name=f"wq{c}") for c in range(EC)]
    wk_sb = [const.tile([P, HD], BF16, tag=f"wk{c}", name=f"wk{c}") for c in range(EC)]
    wo_sb = [const.tile([P, E], BF16, tag=f"wo{c}", name=f"wo{c}") for c in range(EC)]
    bq_t = const.tile([P, NPAIR], F32, tag="bq", name="bq_t")
    bk_t = const.tile([P, NPAIR], F32, tag="bk", name="bk_t")
    bob_t = const.tile([P, E], F32, tag="bob", name="bob_t")
    tri2_t = const.tile([P, 2 * P], BF16, tag="tri2", name="tri2_t")

    # ---- input DMAs: ordered by first use, spread over 3 queues ----
    for c in range(EC):
        nc.sync.dma_start(xt[c][:], d["xt"][c * P:(c + 1) * P, :])
        nc.sync.dma_start(wv_sb[c][:], d["wv"][c * P:(c + 1) * P, :])
    nc.sync.dma_start(bq_t[:], d["bq"][:])
    nc.sync.dma_start(bk_t[:], d["bk"][:])
    for c in range(EC):
        nc.scalar.dma_start(wq_sb[c][:], d["wq"][c * P:(c + 1) * P, :])
    for c in range(EC):
        nc.scalar.dma_start(wk_sb[c][:], d["wk"][c * P:(c + 1) * P, :])
    nc.gpsimd.dma_start(tri2_t[:], d["tri2"][:])
    for c in range(EC):
        nc.gpsimd.dma_start(wo_sb[c][:], d["wo"][c * P:(c + 1) * P, :])
    nc.gpsimd.dma_start(bob_t[:], d["bob"][:])

    # ---- persistent SBUF results ----
    # vd[j]: [128 k, 2 x (8 heads x 65)] bf16, ones column per head
    vd = [const.tile([P, 2 * 520], BF16, tag=f"vd{j}", name=f"vd{j}") for j in range(4)]
    for j in range(4):
        nc.vector.memset(vd[j][:], 1.0)
    qt = {p: const.tile([P, 1024], BF16, tag=f"qt{p}", name=f"qt{p}")
          for p in range(NPAIR)}
    kt = {p: const.tile([P, 1024], BF16, tag=f"kt{p}", name=f"kt{p}")
          for p in range(NPAIR)}
    ot_sb = {}

    def v_copy(j, sh, src):
        t3o = vd[j][:, sh * 520:(sh + 1) * 520].rearrange("p (h c) -> p h c", c=65)
        nc.vector.tensor_copy(
            t3o[:, :, 0:DV], src.rearrange("p (h c) -> p h c", c=DV))

    # ---- upfront projections (paired [128,1024] PSUM tiles, no stalls):
    # V for si 0..3 (k-blocks 0..3) and Q/K for pair 0, q-chunk 0 ----
    for j in range(2):
        vp = ps.tile([P, 1024], F32, tag="st", name=f"vp{j}")
        for sh in range(2):
            si = 2 * j + sh
            for c in range(EC):
                nc.tensor.matmul(
                    vp[:, sh * 512:(sh + 1) * 512],
                    xt[c][:, si * P:(si + 1) * P], wv_sb[c][:],
                    start=(c == 0), stop=(c == EC - 1))
        for sh in range(2):
            v_copy(j, sh, vp[:, sh * 512:(sh + 1) * 512])

    qkp = ps.tile([P, 1024], F32, tag="st", name="qkp0")
    for c in range(EC):
        nc.tensor.matmul(qkp[:, 0:512], wq_sb[c][:, 0:P], xt[c][:, 0:512],
                         start=(c == 0), stop=(c == EC - 1))
    for c in range(EC):
        nc.tensor.matmul(qkp[:, 512:1024], wk_sb[c][:, 0:P], xt[c][:, 0:512],
                         start=(c == 0), stop=(c == EC - 1))
    nc.scalar.add(qt[0][:, 0:512], qkp[:, 0:512], bq_t[:, 0:1])
    nc.vector.tensor_scalar_add(kt[0][:, 0:512], qkp[:, 512:1024], bk_t[:, 0:1])

    # ---- filler units: 4 matmuls + 1 convert each, PSUM tag "yw" ----
    def unit_qk(p, qc, which):
        def emit():
            t = ps.tile([P, 512], F32, tag="yw", bufs=1, name=f"u{which}{p}{qc}")
            w = wq_sb if which == "q" else wk_sb
            for c in range(EC):
                nc.tensor.matmul(
                    t[:], w[c][:, p * P:(p + 1) * P],
                    xt[c][:, qc * 512:(qc + 1) * 512],
                    start=(c == 0), stop=(c == EC - 1))
            if which == "q":
                nc.scalar.add(qt[p][:, qc * 512:(qc + 1) * 512], t[:], bq_t[:, p:p + 1])
            else:
                nc.vector.tensor_scalar_add(
                    kt[p][:, qc * 512:(qc + 1) * 512], t[:], bk_t[:, p:p + 1])
        return emit

    def unit_v(j, sh):
        def emit():
            si = 2 * j + sh
            t = ps.tile([P, 512], F32, tag="yw", bufs=1, name=f"uv{si}")
            for c in range(EC):
                nc.tensor.matmul(t[:], xt[c][:, si * P:(si + 1) * P], wv_sb[c][:],
                                 start=(c == 0), stop=(c == EC - 1))
            v_copy(j, sh, t[:])
        return emit

    def unit_yp(qc, sj):
        def emit():
            si = qc * 4 + sj
            sl = si % 4
            t = ps.tile([P, 512], F32, tag="yw", bufs=1, name=f"uy{si}")
            for p in range(NPAIR):
                nc.tensor.matmul(
                    t[:], ot_sb[p, qc][:, sl * P:(sl + 1) * P], wo_sb[p][:],
                    start=(p == 0), stop=(p == NPAIR - 1))
            yo = work.tile([P, E], F32, tag="yo", name=f"yo{si}", bufs=2)
            nc.vector.tensor_add(yo[:], t[:], bob_t[:])
            nc.sync.dma_start(d["y"][si * P:(si + 1) * P, :], yo[:])
        return emit

    # ---- attention for one (head-pair, q-chunk); fills[ki] emitted between
    # the k-block's scores and the PREVIOUS block's AV (latency hiding) ----
    def attn(p, qc, fills):
        n_ki = 4 * (qc + 1)
        otp = {}
        for hb in range(2):
            otp[hb] = ps.tile([DV + 1, 512], F32, tag="ot", bufs=3,
                              name=f"otp{p}_{qc}_{hb}")
        stes = {}

        def emit_av(ki):
            off = max(ki * P - qc * 512, 0)
            st_f, sp_f = (ki == 0), (ki == n_ki - 1)
            for hb in range(2):
                h = 2 * p + hb
                vsl = (ki % 2) * 520 + h * 65
                nc.tensor.matmul(
                    otp[hb][:, off:], vd[ki // 2][:, vsl:vsl + 65],
                    stes[ki][:, hb * 512 + off:(hb + 1) * 512],
                    start=st_f, stop=sp_f, skip_group_check=True)

        pend = None
        for ki in range(n_ki):
            diag = (ki * P - qc * 512) >= 0
            off = max(ki * P - qc * 512, 0)
            stp = ps.tile([P, 1024], F32, tag="st", name=f"st{p}_{qc}_{ki}")
            for hb in range(2):
                hp = slice(hb * DK, (hb + 1) * DK)
                nc.tensor.matmul(
                    stp[:, hb * 512 + off:(hb + 1) * 512],
                    kt[p][hp, ki * P:(ki + 1) * P],
                    qt[p][hp, qc * 512 + off:(qc + 1) * 512],
                    start=True, stop=True, tile_position=(hb * DK, 0),
                    skip_group_check=True)
            for u in fills.get(ki, []):
                u()
            if pend is not None:
                emit_av(pend)
            ste = work.tile([P, 1024], BF16, tag="ste", name=f"ste{p}_{qc}_{ki}")
            stes[ki] = ste
            stp3 = stp.rearrange("p (h q) -> p h q", h=2)[:, :, off:]
            ste3 = ste.rearrange("p (h q) -> p h q", h=2)[:, :, off:]
            nc.scalar.activation(
                ste3, stp3, mybir.ActivationFunctionType.Exp, scale=0.125)
            if diag:
                nc.vector.tensor_mul(
                    ste3[:, :, 0:P], ste3[:, :, 0:P],
                    tri2_t.rearrange("p (h q) -> p h q", h=2))
            pend = ki
        emit_av(pend)

        # normalize: O^T *= 1/denominator (row DV of each accumulator)
        ot = const.tile([P, 512], BF16, tag=f"ot{p}_{qc}", name=f"ot{p}_{qc}")
        for hb in range(2):
            h = 2 * p + hb
            rrow = work.tile([1, 512], F32, tag="rrow", name=f"rrow{h}_{qc}", bufs=2)
            nc.vector.tensor_copy(rrow[:], otp[hb][DV:DV + 1, :])
            rec = work.tile([1, 512], F32, tag="rec", name=f"rec{h}_{qc}", bufs=2)
            nc.vector.reciprocal_approx_fast(rec[:], rrow[:])
            rb = work.tile([DV, 512], F32, tag="rb", name=f"rb{h}_{qc}", bufs=2)
            nc.gpsimd.partition_broadcast(rb[:], rec[:])
            if hb == 0:
                nc.vector.tensor_mul(ot[0:DV, :], otp[0][0:DV, :], rb[:])
            else:
                # DVE cannot shift partitions: scale into a temp at base 0,
                # then SBUF->SBUF DMA into partitions 64-127 of the pair tile
                tmp = work.tile([DV, 512], BF16, tag="ottmp",
                                name=f"ottmp{p}_{qc}", bufs=2)
                nc.vector.tensor_mul(tmp[:], otp[1][0:DV, :], rb[:])
                nc.sync.dma_start(ot[DV:P, :], tmp[:])
        ot_sb[p, qc] = ot

    # ---- schedule: attention with projection/output fillers threaded in ----
    attn(0, 0, {1: [unit_qk(1, 0, "q")], 3: [unit_qk(1, 0, "k")]})
    attn(1, 0, {1: [unit_qk(2, 0, "q")], 3: [unit_qk(2, 0, "k")]})
    attn(2, 0, {1: [unit_qk(3, 0, "q")], 3: [unit_qk(3, 0, "k")]})
    attn(3, 0, {1: [unit_qk(0, 1, "q")], 3: [unit_qk(0, 1, "k")]})
    attn(0, 1, {0: [unit_v(2, 0)], 1: [unit_v(2, 1)], 2: [unit_v(3, 0)],
                3: [unit_v(3, 1)], 5: [unit_qk(1, 1, "q")],
                7: [unit_qk(1, 1, "k")]})
    attn(1, 1, {2: [unit_qk(2, 1, "q")], 5: [unit_qk(2, 1, "k")]})
    attn(2, 1, {1: [unit_qk(3, 1, "q")], 3: [unit_qk(3, 1, "k")],
                5: [unit_yp(0, 0)], 7: [unit_yp(0, 1)]})
    attn(3, 1, {2: [unit_yp(0, 2)], 5: [unit_yp(0, 3)]})
    for sj in range(4):
        unit_yp(1, sj)()


def _build():
    nc = bacc.Bacc("TRN2", target_bir_lowering=False, debug=False)
    d = {
        "xt": nc.dram_tensor("xt", [E, S], BF16, kind="ExternalInput").ap(),
        "wq": nc.dram_tensor("wq", [E, HD], BF16, kind="ExternalInput").ap(),
        "wk": nc.dram_tensor("wk", [E, HD], BF16, kind="ExternalInput").ap(),
        "wv": nc.dram_tensor("wv", [E, HD], BF16, kind="ExternalInput").ap(),
        "wo": nc.dram_tensor("wo", [HD, E], BF16, kind="ExternalInput").ap(),
        "tri2": nc.dram_tensor("tri2", [P, 2 * P], BF16, kind="ExternalInput").ap(),
        "bq": nc.dram_tensor("bq", [P, NPAIR], F32, kind="ExternalInput").ap(),
        "bk": nc.dram_tensor("bk", [P, NPAIR], F32, kind="ExternalInput").ap(),
        "bob": nc.dram_tensor("bob", [P, E], F32, kind="ExternalInput").ap(),
        "y": nc.dram_tensor("y", [S, E], F32, kind="ExternalOutput").ap(),
    }
    with tile.TileContext(nc) as tc:
        with tc.tile_pool(name="const", bufs=1) as const, \
             tc.tile_pool(name="work", bufs=3) as work, \
             tc.tile_pool(name="ps", bufs=2, space="PSUM") as ps:
            _body(nc, tc, const, work, ps, d)
    nc.compile()
    return nc


def get_nc():
    global _COMPILED
    if _COMPILED is None:
        _COMPILED = _build()
    return _COMPILED


def _prep_in_maps(X, Wq, bq, Wk, bk, Wv, bv, Wo, bo):
    f = np.float32
    bf = ml_dtypes.bfloat16
    Wof = np.asarray(Wo, f).reshape(HD, E)
    # A@(V + 1 bv^T)/d = A@V/d + bv exactly (the ones-column denominator
    # divides out), so bv contributes bv_concat @ Wo to every output row.
    bo_eff = np.asarray(bo, f).reshape(E) + np.asarray(bv, f).reshape(HD) @ Wof
    shared = {
        "wq": np.ascontiguousarray(
            np.transpose(np.asarray(Wq, f), (1, 0, 2)).reshape(E, HD).astype(bf)),
        "wk": np.ascontiguousarray(
            np.transpose(np.asarray(Wk, f), (1, 0, 2)).reshape(E, HD).astype(bf)),
        "wv": np.ascontiguousarray(
            np.transpose(np.asarray(Wv, f), (1, 0, 2)).reshape(E, HD).astype(bf)),
        "wo": np.ascontiguousarray(Wof.astype(bf)),
        "bq": np.ascontiguousarray(np.asarray(bq, f).reshape(HD).reshape(NPAIR, P).T),
        "bk": np.ascontiguousarray(np.asarray(bk, f).reshape(HD).reshape(NPAIR, P).T),
        "bob": np.ascontiguousarray(np.broadcast_to(bo_eff.reshape(1, E), (P, E))),
    }
    # 0/1 keep-mask for the diagonal 128x128 triangle (keep k <= q), twice
    # side by side so one DVE op covers both heads
    keep = np.triu(np.ones((P, P), dtype=f))
    shared["tri2"] = np.ascontiguousarray(np.tile(keep, (1, 2)).astype(bf))
    Xf = np.asarray(X, f)
    in_maps = []
    for b in range(B):
        m = dict(shared)
        m["xt"] = np.ascontiguousarray(Xf[b].T.astype(bf))
        in_maps.append(m)
    return in_maps


def kernel(X, Wq, bq, Wk, bk, Wv, bv, Wo, bo):
    nc = get_nc()
    in_maps = _prep_in_maps(X, Wq, bq, Wk, bk, Wv, bv, Wo, bo)
    res = bass_utils.run_bass_kernel_spmd(nc, in_maps, core_ids=list(range(NCORES)))
    return np.stack([res.results[b]["y"] for b in range(B)], axis=0).astype(np.float32)


def run_traced(X, Wq, bq, Wk, bk, Wv, bv, Wo, bo):
    """Like kernel() but with NTFF profiling; returns (out, exec_time_ns)."""
    nc = get_nc()
    in_maps = _prep_in_maps(X, Wq, bq, Wk, bk, Wv, bv, Wo, bo)
    res = bass_utils.run_bass_kernel_spmd(
        nc, in_maps, core_ids=list(range(NCORES)), trace=True)
    out = np.stack([res.results[b]["y"] for b in range(B)], axis=0).astype(np.float32)
    return out, res.exec_time_ns


# revision 11
# speedup vs baseline: 1.2225x; 1.0744x over previous
"""Multi-head causal attention (B=8, S=1024, E=512, H=8, Dk=Dv=64) on 8 NeuronCores.

Sharding: data-parallel over batch. Core b computes the full attention block
for X[b]; no collectives. Host pre-transposes X[b] -> [E, S], converts matmul
operands to bf16, and pre-arranges weights so the device kernel is pure
matmul + softmax.

v3 structure:
  - PE warmup matmuls at t=0 flip the HAM clock-gate to 8/8 during input DMA;
    input DMAs are ordered by first use and spread across sync/scalar/gpsimd
    descriptor queues.
  - The attention inner loop is ACT(exp)-bound, so projection work is
    interleaved INTO it as "filler units" (4 matmuls + one PSUM->SBUF convert
    each, all [128,512]) emitted between a k-block's scores and the previous
    block's AV matmuls.  This keeps TensorE dense (no HAM re-throttle) and
    software-pipelines the exp latency.
  - bv is folded into the output bias on the host (A@(V + 1 bv^T)/d = A@V/d +
    bv exactly, since the ones-column denominator divides out), so the V
    convert is a plain copy.
  - Causal diagonal: the exp'd triangle is zeroed by one DVE multiply with a
    0/1 slab per diagonal block (no PE mask matmuls).
  - PSUM: tag "st" 2x[128,1024] (scores double-buffer + upfront projection
    pairs), tag "ot" 3x[65,512] (AV accumulators overlap the normalize
    chain across head-pairs), tag "yw" 1x[128,512] (warmup + filler units).

Per-core dataflow (bf16 matmuls, fp32 PSUM accumulate / softmax math):
  V = X @ Wv with a ones column per head (AV emits softmax denominators)
  QT/KT = (W^T X)^T per head-pair -> [128 dd, 1024 q] (+bq/bk per partition)
  per head-pair, q-chunk, k-block: scores^T via two row-tiled K=64 matmuls,
    exp on ScalarE (scale=1/8), DVE triangle zero on diagonal blocks,
    O^T accum = V^T @ exp (65th row = denominator), O^T *= 1/denom via
    fast-NR reciprocal + gpsimd partition broadcast.
  Y[s-chunk] = sum_p O^T-chunk^T @ Wo_p + (bo + bv@Wo), one DMA per 128 rows.
"""

import numpy as np
import ml_dtypes

import concourse.bass as bass
import concourse.tile as tile
import concourse.mybir as mybir
from concourse import bacc
from concourse import bass_utils

B, S, E = 8, 1024, 512
H, DK, DV = 8, 64, 64
HD = H * DK  # 512
P = 128
EC = E // P  # 4 contraction chunks over E
NPAIR = H // 2
NCORES = 8
F32 = mybir.dt.float32
BF16 = mybir.dt.bfloat16

_COMPILED = None


def _body(nc, tc, const, work, ps, d):
    # ---- PE warmup: TensorE busy from t=0 so HAM goes 8/8 during input DMA ----
    wu_src = const.tile([P, 512], BF16, tag="wusrc", name="wu_src")
    nc.vector.memset(wu_src[:], 0.125)
    for i in range(10):
        wu = ps.tile([P, 512], F32, tag="yw", bufs=1, name=f"wu{i}")
        nc.tensor.matmul(wu[:], wu_src[:, 0:P], wu_src[:],
                         start=True, stop=True, skip_group_check=True)

    # ---- SBUF tiles for inputs ----
    xt = [const.tile([P, S], BF16, tag=f"xt{c}", name=f"xt{c}") for c in range(EC)]
    wv_sb = [const.tile([P, HD], BF16, tag=f"wv{c}", name=f"wv{c}") for c in range(EC)]
    wq_sb = [const.tile([P, HD], BF16, tag=f"wq{c}", name=f"wq{c}") for c in range(EC)]
    wk_sb = [const.tile([P, HD], BF16, tag=f"wk{c}", name=f"wk{c}") for c in range(EC)]
    wo_sb = [const.tile([P, E], BF16, tag=f"wo{c}", name=f"wo{c}") for c in range(EC)]
    bq_t = const.tile([P, NPAIR], F32, tag="bq", name="bq_t")
    bk_t = const.tile([P, NPAIR], F32, tag="bk", name="bk_t")
    bob_t = const.tile([P, E], F32, tag="bob", name="bob_t")
    tri2_t = const.tile([P, 2 * P], BF16, tag="tri2", name="tri2_t")

    # ---- input DMAs: ordered by first use, spread over 3 queues ----
    for c in range(EC):
        nc.sync.dma_start(xt[c][:], d["xt"][c * P:(c + 1) * P, :])
        nc.sync.dma_start(wv_sb[c][:], d["wv"][c * P:(c + 1) * P, :])
    nc.sync.dma_start(bq_t[:], d["bq"][:])
    nc.sync.dma_start(bk_t[:], d["bk"][:])
    for c in range(EC):
        nc.scalar.dma_start(wq_sb[c][:], d["wq"][c * P:(c + 1) * P, :])
    for c in range(EC):
        nc.scalar.dma_start(wk_sb[c][:], d["wk"][c * P:(c + 1) * P, :])
    nc.gpsimd.dma_start(tri2_t[:], d["tri2"][:])
    for c in range(EC):
        nc.gpsimd.dma_start(wo_sb[c][:], d["wo"][c * P:(c + 1) * P, :])
    nc.gpsimd.dma_start(bob_t[:], d["bob"][:])

    # ---- persistent SBUF results ----
    # vd[j]: [128 k, 2 x (8 heads x 65)] bf16, ones column per head
    vd = [const.tile([P, 2 * 520], BF16, tag=f"vd{j}", name=f"vd{j}") for j in range(4)]
    for j in range(4):
        nc.vector.memset(vd[j][:], 1.0)
    qt = {p: const.tile([P, 1024], BF16, tag=f"qt{p}", name=f"qt{p}")
          for p in range(NPAIR)}
    kt = {p: const.tile([P, 1024], BF16, tag=f"kt{p}", name=f"kt{p}")
          for p in range(NPAIR)}
    ot_sb = {}

    def v_copy(j, sh, src):
        t3o = vd[j][:, sh * 520:(sh + 1) * 520].rearrange("p (h c) -> p h c", c=65)
        nc.vector.tensor_copy(
            t3o[:, :, 0:DV], src.rearrange("p (h c) -> p h c", c=DV))

    # ---- upfront projections (paired [128,1024] PSUM tiles, no stalls):
    # V for si 0..3 (k-blocks 0..3) and Q/K for pair 0, q-chunk 0 ----
    def v_pair(j):
        vp = ps.tile([P, 1024], F32, tag="st", name=f"vp{j}")
        for sh in range(2):
            si = 2 * j + sh
            for c in range(EC):
                nc.tensor.matmul(
                    vp[:, sh * 512:(sh + 1) * 512],
                    xt[c][:, si * P:(si + 1) * P], wv_sb[c][:],
                    start=(c == 0), stop=(c == EC - 1))
        for sh in range(2):
            v_copy(j, sh, vp[:, sh * 512:(sh + 1) * 512])

    v_pair(0)
    qkp = ps.tile([P, 1024], F32, tag="st", name="qkp0")
    for c in range(EC):
        nc.tensor.matmul(qkp[:, 0:512], wq_sb[c][:, 0:P], xt[c][:, 0:512],
                         start=(c == 0), stop=(c == EC - 1))
    for c in range(EC):
        nc.tensor.matmul(qkp[:, 512:1024], wk_sb[c][:, 0:P], xt[c][:, 0:512],
                         start=(c == 0), stop=(c == EC - 1))
    nc.scalar.add(qt[0][:, 0:512], qkp[:, 0:512], bq_t[:, 0:1])
    nc.vector.tensor_scalar_add(kt[0][:, 0:512], qkp[:, 512:1024], bk_t[:, 0:1])
    v_pair(1)

    # ---- filler units: 4 matmuls + 1 convert each, PSUM tag "yw" ----
    def unit_qk(p, qc, which):
        def emit():
            t = ps.tile([P, 512], F32, tag="yw", bufs=1, name=f"u{which}{p}{qc}")
            w = wq_sb if which == "q" else wk_sb
            for c in range(EC):
                nc.tensor.matmul(
                    t[:], w[c][:, p * P:(p + 1) * P],
                    xt[c][:, qc * 512:(qc + 1) * 512],
                    start=(c == 0), stop=(c == EC - 1))
            if which == "q":
                nc.scalar.add(qt[p][:, qc * 512:(qc + 1) * 512], t[:], bq_t[:, p:p + 1])
            else:
                nc.vector.tensor_scalar_add(
                    kt[p][:, qc * 512:(qc + 1) * 512], t[:], bk_t[:, p:p + 1])
        return emit

    def unit_v(j, sh):
        def emit():
            si = 2 * j + sh
            t = ps.tile([P, 512], F32, tag="yw", bufs=1, name=f"uv{si}")
            for c in range(EC):
                nc.tensor.matmul(t[:], xt[c][:, si * P:(si + 1) * P], wv_sb[c][:],
                                 start=(c == 0), stop=(c == EC - 1))
            v_copy(j, sh, t[:])
        return emit

    def unit_yp(qc, sj):
        def emit():
            si = qc * 4 + sj
            sl = si % 4
            t = ps.tile([P, 512], F32, tag="yw", bufs=1, name=f"uy{si}")
            for p in range(NPAIR):
                nc.tensor.matmul(
                    t[:], ot_sb[p, qc][:, sl * P:(sl + 1) * P], wo_sb[p][:],
                    start=(p == 0), stop=(p == NPAIR - 1))
            yo = work.tile([P, E], F32, tag="yo", name=f"yo{si}", bufs=2)
            nc.vector.tensor_add(yo[:], t[:], bob_t[:])
            nc.sync.dma_start(d["y"][si * P:(si + 1) * P, :], yo[:])
        return emit

    # ---- attention for one (head-pair, q-chunk); fills[ki] emitted between
    # the k-block's scores and the PREVIOUS block's AV (latency hiding) ----
    def attn(p, qc, fills):
        n_ki = 4 * (qc + 1)
        otp = {}
        for hb in range(2):
            otp[hb] = ps.tile([DV + 1, 512], F32, tag="ot", bufs=3,
                              name=f"otp{p}_{qc}_{hb}")
        stes = {}

        def emit_av(ki):
            off = max(ki * P - qc * 512, 0)
            st_f, sp_f = (ki == 0), (ki == n_ki - 1)
            for hb in range(2):
                h = 2 * p + hb
                vsl = (ki % 2) * 520 + h * 65
                nc.tensor.matmul(
                    otp[hb][:, off:], vd[ki // 2][:, vsl:vsl + 65],
                    stes[ki][:, hb * 512 + off:(hb + 1) * 512],
                    start=st_f, stop=sp_f, skip_group_check=True)

        pend = None
        for ki in range(n_ki):
            diag = (ki * P - qc * 512) >= 0
            off = max(ki * P - qc * 512, 0)
            stp = ps.tile([P, 1024], F32, tag="st", name=f"st{p}_{qc}_{ki}")
            for hb in range(2):
                hp = slice(hb * DK, (hb + 1) * DK)
                nc.tensor.matmul(
                    stp[:, hb * 512 + off:(hb + 1) * 512],
                    kt[p][hp, ki * P:(ki + 1) * P],
                    qt[p][hp, qc * 512 + off:(qc + 1) * 512],
                    start=True, stop=True, tile_position=(hb * DK, 0),
                    skip_group_check=True)
            for u in fills.get(ki, []):
                u()
            if pend is not None:
                emit_av(pend)
            ste = work.tile([P, 1024], BF16, tag="ste", name=f"ste{p}_{qc}_{ki}")
            stes[ki] = ste
            stp3 = stp.rearrange("p (h q) -> p h q", h=2)[:, :, off:]
            ste3 = ste.rearrange("p (h q) -> p h q", h=2)[:, :, off:]
            nc.scalar.activation(
                ste3, stp3, mybir.ActivationFunctionType.Exp, scale=0.125)
            if diag:
                nc.vector.tensor_mul(
                    ste3[:, :, 0:P], ste3[:, :, 0:P],
                    tri2_t.rearrange("p (h q) -> p h q", h=2))
            pend = ki
        emit_av(pend)

        # normalize: O^T *= 1/denominator (row DV of each accumulator)
        ot = const.tile([P, 512], BF16, tag=f"ot{p}_{qc}", name=f"ot{p}_{qc}")
        for hb in range(2):
            h = 2 * p + hb
            rrow = work.tile([1, 512], F32, tag="rrow", name=f"rrow{h}_{qc}", bufs=2)
            nc.vector.tensor_copy(rrow[:], otp[hb][DV:DV + 1, :])
            rec = work.tile([1, 512], F32, tag="rec", name=f"rec{h}_{qc}", bufs=2)
            nc.vector.reciprocal_approx_fast(rec[:], rrow[:])
            rb = work.tile([DV, 512], F32, tag="rb", name=f"rb{h}_{qc}", bufs=2)
            nc.gpsimd.partition_broadcast(rb[:], rec[:])
            if hb == 0:
                nc.vector.tensor_mul(ot[0:DV, :], otp[0][0:DV, :], rb[:])
            else:
                # DVE cannot shift partitions: scale into a temp at base 0,
                # then SBUF->SBUF DMA into partitions 64-127 of the pair tile
                tmp = work.tile([DV, 512], BF16, tag="ottmp",
                                name=f"ottmp{p}_{qc}", bufs=2)
                nc.vector.tensor_mul(tmp[:], otp[1][0:DV, :], rb[:])
                nc.sync.dma_start(ot[DV:P, :], tmp[:])
        ot_sb[p, qc] = ot

    # ---- schedule: attention with projection/output fillers threaded in ----
    attn(0, 0, {0: [unit_qk(1, 0, "q")], 2: [unit_qk(1, 0, "k")]})
    attn(1, 0, {0: [unit_qk(2, 0, "q")], 2: [unit_qk(2, 0, "k")]})
    attn(2, 0, {0: [unit_qk(3, 0, "q")], 2: [unit_qk(3, 0, "k")]})
    attn(3, 0, {0: [unit_qk(0, 1, "q")], 2: [unit_qk(0, 1, "k")]})
    attn(0, 1, {0: [unit_v(2, 0)], 1: [unit_v(2, 1)], 2: [unit_v(3, 0)],
                3: [unit_v(3, 1)], 5: [unit_qk(1, 1, "q")],
                7: [unit_qk(1, 1, "k")]})
    attn(1, 1, {2: [unit_qk(2, 1, "q")], 5: [unit_qk(2, 1, "k")]})
    attn(2, 1, {1: [unit_qk(3, 1, "q")], 3: [unit_qk(3, 1, "k")],
                5: [unit_yp(0, 0)], 7: [unit_yp(0, 1)]})
    attn(3, 1, {2: [unit_yp(0, 2)], 5: [unit_yp(0, 3)]})
    # final output projection (si 4..7): two paired PSUM tiles; pairs 0-2
    # accumulate while pair (3,1)'s normalize chain runs, pair 3 lands last
    yps = []
    for g in range(2):
        yp = ps.tile([P, 1024], F32, tag="st", name=f"ypf{g}")
        for sh in range(2):
            sl = g * 2 + sh
            for p in range(3):
                nc.tensor.matmul(
                    yp[:, sh * 512:(sh + 1) * 512],
                    ot_sb[p, 1][:, sl * P:(sl + 1) * P], wo_sb[p][:],
                    start=(p == 0), stop=False, skip_group_check=True)
        yps.append(yp)
    for g in range(2):
        for sh in range(2):
            sl = g * 2 + sh
            nc.tensor.matmul(
                yps[g][:, sh * 512:(sh + 1) * 512],
                ot_sb[3, 1][:, sl * P:(sl + 1) * P], wo_sb[3][:],
                start=False, stop=True, skip_group_check=True)
        yo = work.tile([P, 1024], F32, tag="yof", name=f"yof{g}", bufs=2)
        for sh in range(2):
            nc.vector.tensor_add(
                yo[:, sh * 512:(sh + 1) * 512],
                yps[g][:, sh * 512:(sh + 1) * 512], bob_t[:])
        r0 = (4 + g * 2) * P
        yv = d["y"][r0:r0 + 2 * P, :].rearrange("(s p) e -> p s e", s=2)
        nc.sync.dma_start(yv, yo.rearrange("p (s e) -> p s e", s=2))


def _build():
    nc = bacc.Bacc("TRN2", target_bir_lowering=False, debug=False)
    d = {
        "xt": nc.dram_tensor("xt", [E, S], BF16, kind="ExternalInput").ap(),
        "wq": nc.dram_tensor("wq", [E, HD], BF16, kind="ExternalInput").ap(),
        "wk": nc.dram_tensor("wk", [E, HD], BF16, kind="ExternalInput").ap(),
        "wv": nc.dram_tensor("wv", [E, HD], BF16, kind="ExternalInput").ap(),
        "wo": nc.dram_tensor("wo", [HD, E], BF16, kind="ExternalInput").ap(),
        "tri2": nc.dram_tensor("tri2", [P, 2 * P], BF16, kind="ExternalInput").ap(),
        "bq": nc.dram_tensor("bq", [P, NPAIR], F32, kind="ExternalInput").ap(),
        "bk": nc.dram_tensor("bk", [P, NPAIR], F32, kind="ExternalInput").ap(),
        "bob": nc.dram_tensor("bob", [P, E], F32, kind="ExternalInput").ap(),
        "y": nc.dram_tensor("y", [S, E], F32, kind="ExternalOutput").ap(),
    }
    with tile.TileContext(nc) as tc:
        with tc.tile_pool(name="const", bufs=1) as const, \
             tc.tile_pool(name="work", bufs=3) as work, \
             tc.tile_pool(name="ps", bufs=2, space="PSUM") as ps:
            _body(nc, tc, const, work, ps, d)
    nc.compile()
    return nc


def get_nc():
    global _COMPILED
    if _COMPILED is None:
        _COMPILED = _build()
    return _COMPILED


def _prep_in_maps(X, Wq, bq, Wk, bk, Wv, bv, Wo, bo):
    f = np.float32
    bf = ml_dtypes.bfloat16
    Wof = np.asarray(Wo, f).reshape(HD, E)
    # A@(V + 1 bv^T)/d = A@V/d + bv exactly (the ones-column denominator
    # divides out), so bv contributes bv_concat @ Wo to every output row.
    bo_eff = np.asarray(bo, f).reshape(E) + np.asarray(bv, f).reshape(HD) @ Wof
    shared = {
        "wq": np.ascontiguousarray(
            np.transpose(np.asarray(Wq, f), (1, 0, 2)).reshape(E, HD).astype(bf)),
        "wk": np.ascontiguousarray(
            np.transpose(np.asarray(Wk, f), (1, 0, 2)).reshape(E, HD).astype(bf)),
        "wv": np.ascontiguousarray(
            np.transpose(np.asarray(Wv, f), (1, 0, 2)).reshape(E, HD).astype(bf)),
        "wo": np.ascontiguousarray(Wof.astype(bf)),
        "bq": np.ascontiguousarray(np.asarray(bq, f).reshape(HD).reshape(NPAIR, P).T),
        "bk": np.ascontiguousarray(np.asarray(bk, f).reshape(HD).reshape(NPAIR, P).T),
        "bob": np.ascontiguousarray(np.broadcast_to(bo_eff.reshape(1, E), (P, E))),
    }
    # 0/1 keep-mask for the diagonal 128x128 triangle (keep k <= q), twice
    # side by side so one DVE op covers both heads
    keep = np.triu(np.ones((P, P), dtype=f))
    shared["tri2"] = np.ascontiguousarray(np.tile(keep, (1, 2)).astype(bf))
    Xf = np.asarray(X, f)
    in_maps = []
    for b in range(B):
        m = dict(shared)
        m["xt"] = np.ascontiguousarray(Xf[b].T.astype(bf))
        in_maps.append(m)
    return in_maps


def kernel(X, Wq, bq, Wk, bk, Wv, bv, Wo, bo):
    nc = get_nc()
    in_maps = _prep_in_maps(X, Wq, bq, Wk, bk, Wv, bv, Wo, bo)
    res = bass_utils.run_bass_kernel_spmd(nc, in_maps, core_ids=list(range(NCORES)))
    return np.stack([res.results[b]["y"] for b in range(B)], axis=0).astype(np.float32)


def run_traced(X, Wq, bq, Wk, bk, Wv, bv, Wo, bo):
    """Like kernel() but with NTFF profiling; returns (out, exec_time_ns)."""
    nc = get_nc()
    in_maps = _prep_in_maps(X, Wq, bq, Wk, bk, Wv, bv, Wo, bo)
    res = bass_utils.run_bass_kernel_spmd(
        nc, in_maps, core_ids=list(range(NCORES)), trace=True)
    out = np.stack([res.results[b]["y"] for b in range(B)], axis=0).astype(np.float32)
    return out, res.exec_time_ns
